# revision 40
# baseline (speedup 1.0000x reference)
"""MinGRU block kernel for Trainium2 (Bass/Tile), SPMD over 8 NeuronCores.

Problem: B=8, S=2048, D=1024, F=3072 (nn_MinGRUBlock).
Sharding: data-parallel over batch (one batch row per core); weights replicated.

fp8 edition: the g/d projections, FFN gate/up, and W_out matmuls run as
float8_e4m3 DoubleRow matmuls (2 k-planes per pass); the v projection stays
bf16 (the tanh path dominates the error budget). fp8 weights are pre-scaled
x256 on the host so |w| clears the fp8 subnormal range; the 1/256 is folded
into the ACT activation `scale` when reading PSUM.

Per-core dataflow (compute in "T layout": feature on partitions, time on free):
  phase 1 (mixer, s-chunks of 512):
    x loaded in bf16, PE-transposed -> xT [d,s] bf16
    rms row-sums: squares on GpSimd (fp8 out), PE fp8-DoubleRow ones-reduce,
    sqrt (ACT) + reciprocal_approx_fast (DVE), PE f32 broadcast -> bf16 SBUF
    xnTb = xT * r (DVE bf16, 2x mode); xnT fp8 cast on ACT
    v: bf16 matmuls; g/d: fp8 DoubleRow matmuls; activations on ACT
    xs/a_t computed in place (DVE); h_t = a_t*h + x_t via tensor_tensor_scan
    out1 = x + h (bf16) into a resident [128, KD, S] tile (no DRAM bounce);
    o1n = fp8(out1*r2) on GpSimd, resident
  phase 2 (FFN): wgu streamed, wout resident;
    gate = silu(gps/256) on ACT (Sigmoid+mults when SILU_ON_ACT=False, for
    CoreSim which lacks Silu); z = (ups*16/256)*gate -> fp8 (DVE
    scalar_tensor_tensor); W_out fp8 DoubleRow; residual = (yps/4096 + out1)
    via one DVE scalar_tensor_tensor reading the resident out1; bf16
    PE-transpose back; f32 out.
"""

import os
import sys
from contextlib import ExitStack

import numpy as np
import ml_dtypes

for _p in ("/opt/trn_rl_repo", "/root/.axon_site/_ro/trn_rl_repo"):
    if os.path.isdir(_p) and _p not in sys.path:
        sys.path.insert(0, _p)

import concourse.bass as bass
import concourse.tile as tile
from concourse import bacc, mybir
from concourse.bass_utils import run_bass_kernel_spmd

F32 = mybir.dt.float32
F16 = mybir.dt.float16
BF16 = mybir.dt.bfloat16
FP8 = mybir.dt.float8e4
AF = mybir.ActivationFunctionType
OP = mybir.AluOpType
PM = mybir.MatmulPerfMode.DoubleRow

B, S, D, F = 8, 2048, 1024, 3072
EPS = 1e-6
KD = D // 128          # 8 d-ptiles
NKP = KD // 2          # 4 d-pair tiles
MF2 = 2 * F // 128     # 48 f-ptiles (gate|up)
MFO = F // 128         # 24 f-ptiles
NOP = MFO // 2         # 12 f-pair tiles
MD = D // 128          # 8 d-ptiles (output)

CH = 512               # s-chunk (both phases)
NCH = S // CH          # 4
NST = CH // 128        # 4 s-tiles per chunk

WS = 256.0             # weight fp8 pre-scale
ZS = 16.0              # z fp8 pre-scale

# Hardware ACT has Silu in its function tables; CoreSim does not implement
# it. sim_check.py flips this off to validate structure/numerics in sim.
SILU_ON_ACT = True


def build_program():
    nc = bacc.Bacc("TRN2", target_bir_lowering=False, debug=False)

    x_d = nc.dram_tensor("x", [S, D], BF16, kind="ExternalInput").ap()
    # g/d projections fp8 (x256); v projection bf16 (tanh path dominates error)
    wmix_d = nc.dram_tensor("w_mix", [2 * MD, 128, KD, 128], FP8, kind="ExternalInput").ap()
    wv_d = nc.dram_tensor("w_v", [MD, 128, KD, 128], BF16, kind="ExternalInput").ap()
    bmix_d = nc.dram_tensor("b_mix", [128, 3 * MD], F32, kind="ExternalInput").ap()
    wgu_d = nc.dram_tensor("w_gu", [MF2, 128, KD, 128], FP8, kind="ExternalInput").ap()
    wout_d = nc.dram_tensor("w_out", [MD, 128, MFO, 128], FP8, kind="ExternalInput").ap()
    ident_d = nc.dram_tensor("ident", [128, 128], BF16, kind="ExternalInput").ap()
    ones2_d = nc.dram_tensor("ones2", [128, 2, 16], FP8, kind="ExternalInput").ap()
    out_d = nc.dram_tensor("out", [S, D], F32, kind="ExternalOutput").ap()

    with tile.TileContext(nc) as tc, ExitStack() as top:
        # ---------- persistent tiles ----------
        cpool = top.enter_context(tc.tile_pool(name="consts", bufs=1))
        ident = cpool.tile([128, 128], BF16)
        nc.sync.dma_start(ident[:], ident_d[:])
        ones2 = cpool.tile([128, 2, 16], FP8)
        nc.sync.dma_start(ones2[:], ones2_d[:])
        ones_row = cpool.tile([1, 128], F32)
        nc.vector.memset(ones_row[:], 1.0)
        eps1 = cpool.tile([1, 1], F32)
        nc.vector.memset(eps1[:], EPS)
        bmix = cpool.tile([128, 3 * MD], F32)
        nc.sync.dma_start(bmix[:], bmix_d[:])

        # out1 (x + h) stays resident in SBUF across phase 1 -> 2b (bf16);
        # normalized out1 resident as fp8 for the FFN matmuls.
        res_pool = top.enter_context(tc.tile_pool(name="resid", bufs=1))
        o1all = res_pool.tile([128, KD, S], BF16)
        o1n = res_pool.tile([128, KD, S], FP8)

        carry_pool = top.enter_context(tc.tile_pool(name="carry", bufs=1))
        carry = carry_pool.tile([128, KD], F32)

        # norm helpers outlive phase 1 (the last two norm2 finishes are
        # emitted between the two FFN gate/up passes)
        p_sq = top.enter_context(tc.tile_pool(name="sqbufs", bufs=2))
        p_row = top.enter_context(tc.tile_pool(name="rows", bufs=1))
        p_bcS = top.enter_context(tc.tile_pool(name="bcS", bufs=1))
        ps_bc = top.enter_context(tc.tile_pool(name="bc_ps", bufs=2, space="PSUM"))

        # ---------- phase 1: mixer (software-pipelined over chunks) ----------
        with ExitStack() as ph1:
            wpool = ph1.enter_context(tc.tile_pool(name="wmix", bufs=1))
            wmix = wpool.tile([128, 2 * MD, KD, 128], FP8)
            wmix_dp = wmix_d.rearrange("m p k j -> p m k j")
            wv16 = wpool.tile([128, MD, KD, 128], BF16)
            wv_dp = wv_d.rearrange("m p k j -> p m k j")

            p_xT = ph1.enter_context(tc.tile_pool(name="xT", bufs=3))
            p_x16 = ph1.enter_context(tc.tile_pool(name="x16", bufs=2))
            p_x8 = ph1.enter_context(tc.tile_pool(name="x8", bufs=2))
            p_16 = ph1.enter_context(tc.tile_pool(name="f16bufs", bufs=2))
            p_scan = ph1.enter_context(tc.tile_pool(name="scanbufs", bufs=1))
            ps_mm = ph1.enter_context(tc.tile_pool(name="mm_ps", bufs=3, space="PSUM"))

            st_front = {}   # c -> (xT, rrow1)
            st_bc1 = {}     # c -> bc1 bf16 SBUF row-broadcast tile
            st_body = {}    # c -> (xs, a_t)  [in-place in sig_g / sig_d]
            st_back = {}    # c -> (out1_slice, rrow2)
            st_sq1 = {}

            def front_t(c):
                """load x chunk transposed via the DMA xbar, squares on GpSimd."""
                s0 = c * CH
                xT = p_xT.tile([128, KD, CH], BF16, tag="xT", name=f"xT{c}")
                nc.sync.dma_start_transpose(xT[:], x_d[s0:s0 + CH, :])
                sq = p_sq.tile([128, KD, CH], FP8, tag="sq", name=f"sq1_{c}")
                nc.scalar.activation(sq[:], xT[:], AF.Square)
                st_front[c] = (xT, None)
                st_sq1[c] = sq

            def norm_rows(sq, label):
                """fp8-DoubleRow ones-reduce + sqrt + fast reciprocal."""
                ss = ps_bc.tile([1, CH], F32, tag="bc", name=f"ss{label}")
                for kp in range(NKP):
                    nc.tensor.matmul(ss[:], ones2[:, :, 0:1],
                                     sq[:, 2 * kp:2 * kp + 2, :],
                                     start=(kp == 0), stop=(kp == NKP - 1),
                                     perf_mode=PM)
                srow = p_row.tile([1, CH], F32, tag=f"srow{label[0]}", name=f"srow{label}")
                nc.scalar.activation(srow[:], ss[:], AF.Sqrt, bias=eps1[:], scale=1.0 / D)
                rrow = p_row.tile([1, CH], F32, tag=f"rrow{label[0]}", name=f"rrow{label}")
                nc.vector.reciprocal_approx_fast(rrow[:], srow[:])
                return rrow

            def bcast(rrow, tag, name):
                """PE f32 row-broadcast -> bf16 SBUF copy (DVE)."""
                bc = ps_bc.tile([128, CH], F32, tag="bc", name=f"bcp_{name}")
                nc.tensor.matmul(bc[:], ones_row[:], rrow[:])
                bcS = p_bcS.tile([128, CH], BF16, tag=tag, name=f"bcS_{name}")
                nc.vector.tensor_copy(bcS[:], bc[:])
                return bcS

            st_ss1 = {}

            def front_red(c):
                sq = st_sq1[c]
                ss = ps_bc.tile([1, CH], F32, tag="bc", name=f"ss1_{c}")
                for kp in range(NKP):
                    nc.tensor.matmul(ss[:], ones2[:, :, 0:1],
                                     sq[:, 2 * kp:2 * kp + 2, :],
                                     start=(kp == 0), stop=(kp == NKP - 1),
                                     perf_mode=PM)
                st_ss1[c] = ss

            def front_sqrt(c):
                ss = st_ss1[c]
                srow = p_row.tile([1, CH], F32, tag="srow1", name=f"srow1_{c}")
                nc.scalar.activation(srow[:], ss[:], AF.Sqrt, bias=eps1[:], scale=1.0 / D)
                rrow = p_row.tile([1, CH], F32, tag="rrow1", name=f"rrow1_{c}")
                nc.vector.reciprocal_approx_fast(rrow[:], srow[:])
                st_front[c] = (st_front[c][0], rrow)

            def bcast1(c):
                st_bc1[c] = bcast(st_front[c][1], "bc1", f"1_{c}")

            st_prep = {}

            def prep(c):
                """normalized input: bf16 (DVE) + fp8 cast (ACT), one chunk
                ahead of body so PE never waits on the normalize chain."""
                xT = st_front[c][0]
                bc1 = st_bc1[c]
                xnTb = p_x16.tile([128, KD, CH], BF16, tag="xnTb", name=f"xnTb{c}")
                for kt in range(KD):
                    nc.vector.tensor_tensor(xnTb[:, kt], xT[:, kt], bc1[:], OP.mult)
                st_prep[c] = (xnTb, None)

            def prep_cast(c):
                xnTb, _ = st_prep[c]
                xnT = p_x8.tile([128, KD, CH], FP8, tag="xnT", name=f"xnT{c}")
                nc.vector.tensor_copy(xnT[:], xnTb[:])
                st_prep[c] = (xnTb, xnT)

            def _proj(c, dst, fn, slot, bcol):
                xnTb, xnT = st_prep[c]
                for half in range(MD // 2):
                    ps = ps_mm.tile([128, 2, CH], F32, tag="mm",
                                    name=f"mm{c}_{slot}_{half}")
                    for mi in range(2):
                        m = half * 2 + mi
                        if slot is None:
                            for kt in range(KD):
                                nc.tensor.matmul(ps[:, mi], wv16[:, m, kt, :],
                                                 xnTb[:, kt, :],
                                                 start=(kt == 0),
                                                 stop=(kt == KD - 1))
                        else:
                            for kp in range(NKP):
                                nc.tensor.matmul(ps[:, mi],
                                                 wmix[:, slot + m, 2 * kp:2 * kp + 2, :],
                                                 xnT[:, 2 * kp:2 * kp + 2, :],
                                                 start=(kp == 0), stop=(kp == NKP - 1),
                                                 perf_mode=PM)
                    for mi in range(2):
                        m = half * 2 + mi
                        nc.scalar.activation(dst[:, m], ps[:, mi], fn,
                                             bias=bmix[:, bcol + m:bcol + m + 1],
                                             scale=(1.0 if slot is None else 1.0 / WS))

            def body_v(c):
                tanh_v = p_16.tile([128, MD, CH], F16, tag="tanh_v", bufs=1, name=f"tv{c}")
                _proj(c, tanh_v, AF.Tanh, None, MD)
                st_body[c] = tanh_v

            def body_gd(c):
                tanh_v = st_body[c]
                sig_g = p_16.tile([128, MD, CH], F16, tag="sig_g", name=f"sg{c}")
                sig_d = p_16.tile([128, MD, CH], F16, tag="sig_d", name=f"sd{c}")
                _proj(c, sig_g, AF.Sigmoid, 0, 0)
                _proj(c, sig_d, AF.Sigmoid, MD, 2 * MD)
                st_body[c] = (sig_g, sig_d, tanh_v)

            def xs_at(c):
                # in-place: xs into sig_g, a_t into sig_d (DVE, after prep so
                # the next chunk's normalize never queues behind these)
                sig_g, sig_d, tanh_v = st_body[c]
                nc.vector.tensor_tensor(sig_g[:], sig_g[:], tanh_v[:], OP.mult)
                nc.vector.tensor_scalar(sig_d[:], sig_d[:], 0.998, 0.001, OP.mult, OP.add)
                st_body[c] = (sig_g, sig_d)

            def back_scan(c):
                """scan, residual into resident out1, squares (DVE+Pool only)."""
                xs, a_t = st_body[c]
                xT = st_front[c][0]
                s0 = c * CH
                hT = p_scan.tile([128, KD, CH], F16, tag="hT", name=f"hT{c}")
                for kt in range(KD):
                    init = 0.0 if c == 0 else carry[:, kt:kt + 1]
                    nc.vector.tensor_tensor_scan(hT[:, kt], a_t[:, kt], xs[:, kt],
                                                 init, OP.mult, OP.add)
                if c + 1 < NCH:
                    nc.vector.tensor_copy(carry[:], hT[:, :, CH - 1])
                out1 = o1all[:, :, s0:s0 + CH]
                nc.vector.tensor_tensor(out1, xT[:], hT[:], OP.add)
                sq2 = p_sq.tile([128, KD, CH], FP8, tag="sq", name=f"sq2_{c}")
                nc.gpsimd.tensor_tensor(sq2[:], out1, out1, OP.mult)
                st_back[c] = (out1, sq2)

            def back_fin(c):
                """norm2 reduce/broadcast (inputs ready); fp8 o1n on GpSimd."""
                s0 = c * CH
                out1, sq2 = st_back[c]
                rrow = norm_rows(sq2, f"2_{c}")
                bcS = bcast(rrow, "bc2", f"2_{c}")
                for kt in range(KD):
                    nc.gpsimd.tensor_tensor(o1n[:, kt, s0:s0 + CH],
                                            out1[:, kt], bcS[:], OP.mult)

            # pipelined emission: prep runs one chunk ahead of body (PE never
            # waits on the normalize chain); norm2 finish lags two chunks.
            front_t(0)
            front_t(1)
            for mt in range(MD):
                nc.sync.dma_start(wv16[:, mt], wv_dp[:, mt])
            for mt in range(2 * MD):
                nc.sync.dma_start(wmix[:, mt], wmix_dp[:, mt])
            front_red(0)
            front_sqrt(0)
            bcast1(0)
            prep(0)
            prep_cast(0)
            for c in range(NCH):
                if 2 <= c + 1 < NCH:
                    front_t(c + 1)
                if c >= 2:
                    back_fin(c - 2)
                body_v(c)
                if c + 1 < NCH:
                    front_red(c + 1)
                    front_sqrt(c + 1)
                    bcast1(c + 1)
                    prep(c + 1)
                body_gd(c)
                xs_at(c)
                if c >= 1:
                    back_scan(c - 1)
                if c + 1 < NCH:
                    prep_cast(c + 1)
            back_scan(NCH - 1)

        # ---------- phase 2: FFN ----------
        with ExitStack() as ph2:
            zpool = ph2.enter_context(tc.tile_pool(name="zbuf", bufs=1))
            z = zpool.tile([128, MFO, S], FP8)
            wo_pool = ph2.enter_context(tc.tile_pool(name="wout", bufs=1))
            wout = wo_pool.tile([128, MD, MFO, 128], FP8)
            wout_dp = wout_d.rearrange("m p k j -> p m k j")
            for mo in range(MD):
                nc.sync.dma_start(wout[:, mo], wout_dp[:, mo])

            # 2a: gate/up + z.  sc=3 deferred so PE need not wait on the
            # final o1n chunk while earlier chunks still have work.
            with ExitStack() as ph2a:
                p_wgu = ph2a.enter_context(tc.tile_pool(name="wgu", bufs=6))
                p_g16 = ph2a.enter_context(tc.tile_pool(name="g16", bufs=3))
                ps_gu = ph2a.enter_context(tc.tile_pool(name="gu_ps", bufs=4, space="PSUM"))
                for scs in ([0, 1], [2, 3]):
                    if scs[0] == 2:
                        back_fin(NCH - 2)
                        back_fin(NCH - 1)
                    for mg in range(MFO):
                        wg = p_wgu.tile([128, KD, 128], FP8, tag="wgu")
                        nc.sync.dma_start(wg[:], wgu_d[mg])
                        wu = p_wgu.tile([128, KD, 128], FP8, tag="wgu")
                        nc.sync.dma_start(wu[:], wgu_d[MFO + mg])
                        for sc in scs:
                            sl = slice(sc * CH, (sc + 1) * CH)
                            gps = ps_gu.tile([128, CH], F32, tag="gups")
                            for kp in range(NKP):
                                nc.tensor.matmul(gps[:], wg[:, 2 * kp:2 * kp + 2, :],
                                                 o1n[:, 2 * kp:2 * kp + 2, sl],
                                                 start=(kp == 0), stop=(kp == NKP - 1),
                                                 perf_mode=PM)
                            ups = ps_gu.tile([128, CH], F32, tag="gups")
                            for kp in range(NKP):
                                nc.tensor.matmul(ups[:], wu[:, 2 * kp:2 * kp + 2, :],
                                                 o1n[:, 2 * kp:2 * kp + 2, sl],
                                                 start=(kp == 0), stop=(kp == NKP - 1),
                                                 perf_mode=PM)
                            gate = p_g16.tile([128, CH], F16, tag="gate")
                            if SILU_ON_ACT:
                                nc.scalar.activation(gate[:], gps[:], AF.Silu,
                                                     scale=1.0 / WS)
                            else:
                                sig = p_g16.tile([128, CH], F16, tag="sig")
                                nc.scalar.activation(sig[:], gps[:], AF.Sigmoid,
                                                     scale=1.0 / WS)
                                nc.vector.scalar_tensor_tensor(gate[:], gps[:], 1.0 / WS,
                                                               sig[:], OP.mult, OP.mult)
                            nc.vector.scalar_tensor_tensor(z[:, mg, sl], ups[:], ZS / WS,
                                                           gate[:], OP.mult, OP.mult)

            # 2b: W_out + residual (from resident out1) + transpose out
            with ExitStack() as ph2b:
                p_oT = ph2b.enter_context(tc.tile_pool(name="outT", bufs=MD + 1))
                p_onat = ph2b.enter_context(tc.tile_pool(name="onat", bufs=3))
                ps_y = ph2b.enter_context(tc.tile_pool(name="y_ps", bufs=2, space="PSUM"))
                ps_t2 = ph2b.enter_context(tc.tile_pool(name="t2_ps", bufs=2, space="PSUM"))
                for sc in range(NCH):
                    sl = slice(sc * CH, (sc + 1) * CH)
                    outTs = []
                    for mo in range(MD):
                        yps = ps_y.tile([128, CH], F32, tag="yps")
                        for op_ in range(NOP):
                            nc.tensor.matmul(yps[:], wout[:, mo, 2 * op_:2 * op_ + 2, :],
                                             z[:, 2 * op_:2 * op_ + 2, sl],
                                             start=(op_ == 0), stop=(op_ == NOP - 1),
                                             perf_mode=PM)
                        oT = p_oT.tile([128, CH], BF16, tag="oT")
                        nc.vector.scalar_tensor_tensor(oT[:], yps[:], 1.0 / (WS * ZS),
                                                       o1all[:, mo, sl], OP.mult, OP.add)
                        outTs.append(oT)
                    for q in range(NST):
                        onat = p_onat.tile([128, D], F32, tag="onat")
                        for h in range(2):
                            t2 = ps_t2.tile([128, 512], BF16, tag="t2")
                            for j in range(4):
                                nc.tensor.transpose(
                                    t2[:, j * 128:(j + 1) * 128],
                                    outTs[4 * h + j][:, q * 128:(q + 1) * 128],
                                    ident[:])
                            nc.scalar.copy(onat[:, h * 512:(h + 1) * 512], t2[:])
                        srow0 = sc * CH + q * 128
                        nc.sync.dma_start(out_d[srow0:srow0 + 128, :], onat[:])

    nc.compile()
    return nc


_NC = None


def _get_nc():
    global _NC
    if _NC is None:
        _NC = build_program()
    return _NC


def _q8(a):
    return np.clip(a, -240.0, 240.0).astype(ml_dtypes.float8_e4m3)


def _prep_weights(inputs):
    w1 = np.asarray(inputs["rms_mix_w"], np.float32)
    w2 = np.asarray(inputs["rms_ffn_w"], np.float32)
    Wg = np.asarray(inputs["Wg"], np.float32) * w1[None, :]
    Wv = np.asarray(inputs["Wv"], np.float32) * w1[None, :]
    Wd = np.asarray(inputs["Wd"], np.float32) * w1[None, :]
    Wcat = np.concatenate([Wg, Wd], axis=0) * WS            # [2D, D] fp8 x256
    w_mix = _q8(np.ascontiguousarray(
        Wcat.T.reshape(KD, 128, 2 * MD, 128).transpose(2, 1, 0, 3)))
    w_v = np.ascontiguousarray(
        Wv.T.reshape(KD, 128, MD, 128).transpose(2, 1, 0, 3)).astype(ml_dtypes.bfloat16)
    bcat = np.concatenate([np.asarray(inputs["bg"], np.float32),
                           np.asarray(inputs["bv"], np.float32),
                           np.asarray(inputs["bd"], np.float32)])
    b_mix = np.ascontiguousarray(bcat.reshape(3 * MD, 128).T).astype(np.float32)
    Wgate = np.asarray(inputs["W_gate"], np.float32) * w2[None, :]
    Wup = np.asarray(inputs["W_up"], np.float32) * w2[None, :]
    Wcat2 = np.concatenate([Wgate, Wup], axis=0) * WS       # [2F, D]
    w_gu = _q8(np.ascontiguousarray(
        Wcat2.T.reshape(KD, 128, MF2, 128).transpose(2, 1, 0, 3)))
    WoT = np.asarray(inputs["W_out"], np.float32).T * WS    # [F, D]
    w_out = _q8(np.ascontiguousarray(
        WoT.reshape(MFO, 128, MD, 128).transpose(2, 1, 0, 3)))
    return {
        "w_mix": w_mix, "w_v": w_v, "b_mix": b_mix, "w_gu": w_gu, "w_out": w_out,
        "ident": np.eye(128).astype(ml_dtypes.bfloat16),
        "ones2": np.ones((128, 2, 16)).astype(ml_dtypes.float8_e4m3),
    }


def run(inputs, trace=False, **kw):
    x = np.asarray(inputs["x"], np.float32)
    shared = _prep_weights(inputs)
    in_maps = [
        dict(shared, x=np.ascontiguousarray(x[b]).astype(ml_dtypes.bfloat16))
        for b in range(B)
    ]
    res = run_bass_kernel_spmd(_get_nc(), in_maps, list(range(B)), trace=trace, **kw)
    out = np.stack([np.asarray(res.results[b]["out"], np.float32) for b in range(B)])
    return out, res


def kernel(**inputs) -> np.ndarray:
    out, _ = run(inputs)
    return out


# revision 44
# speedup vs baseline: 1.0109x; 1.0109x over previous
"""MinGRU block kernel for Trainium2 (Bass/Tile), SPMD over 8 NeuronCores.

Problem: B=8, S=2048, D=1024, F=3072 (nn_MinGRUBlock).
Sharding: data-parallel over batch (one batch row per core); weights replicated.

fp8 edition: the g/d projections, FFN gate/up, and W_out matmuls run as
float8_e4m3 DoubleRow matmuls (2 k-planes per pass); the v projection stays
bf16 (the tanh path dominates the error budget). fp8 weights are pre-scaled
x256 on the host so |w| clears the fp8 subnormal range; the 1/256 is folded
into the ACT activation `scale` when reading PSUM.

Per-core dataflow (compute in "T layout": feature on partitions, time on free):
  phase 1 (mixer, s-chunks of 512):
    x loaded in bf16, PE-transposed -> xT [d,s] bf16
    rms row-sums: squares on GpSimd (fp8 out), PE fp8-DoubleRow ones-reduce,
    sqrt (ACT) + reciprocal_approx_fast (DVE), PE f32 broadcast -> bf16 SBUF
    xnTb = xT * r (DVE bf16, 2x mode); xnT fp8 cast on ACT
    v: bf16 matmuls; g/d: fp8 DoubleRow matmuls; activations on ACT
    xs/a_t computed in place (DVE); h_t = a_t*h + x_t via tensor_tensor_scan
    out1 = x + h (bf16) into a resident [128, KD, S] tile (no DRAM bounce);
    o1n = fp8(out1*r2) on GpSimd, resident
  phase 2 (FFN): wgu streamed, wout resident;
    gate = silu(gps/256) on ACT (Sigmoid+mults when SILU_ON_ACT=False, for
    CoreSim which lacks Silu); z = (ups*16/256)*gate -> fp8 (DVE
    scalar_tensor_tensor); W_out fp8 DoubleRow; residual = (yps/4096 + out1)
    via one DVE scalar_tensor_tensor reading the resident out1; bf16
    PE-transpose back; f32 out.
"""

import os
import sys
from contextlib import ExitStack

import numpy as np
import ml_dtypes

for _p in ("/opt/trn_rl_repo", "/root/.axon_site/_ro/trn_rl_repo"):
    if os.path.isdir(_p) and _p not in sys.path:
        sys.path.insert(0, _p)

import concourse.bass as bass
import concourse.tile as tile
from concourse import bacc, mybir
from concourse.bass_utils import run_bass_kernel_spmd

F32 = mybir.dt.float32
F16 = mybir.dt.float16
BF16 = mybir.dt.bfloat16
FP8 = mybir.dt.float8e4
AF = mybir.ActivationFunctionType
OP = mybir.AluOpType
PM = mybir.MatmulPerfMode.DoubleRow

B, S, D, F = 8, 2048, 1024, 3072
EPS = 1e-6
KD = D // 128          # 8 d-ptiles
NKP = KD // 2          # 4 d-pair tiles
MF2 = 2 * F // 128     # 48 f-ptiles (gate|up)
MFO = F // 128         # 24 f-ptiles
NOP = MFO // 2         # 12 f-pair tiles
MD = D // 128          # 8 d-ptiles (output)

CH = 512               # s-chunk (both phases)
NCH = S // CH          # 4
NST = CH // 128        # 4 s-tiles per chunk

WS = 256.0             # weight fp8 pre-scale
ZS = 16.0              # z fp8 pre-scale

# Hardware ACT has Silu in its function tables; CoreSim does not implement
# it. sim_check.py flips this off to validate structure/numerics in sim.
SILU_ON_ACT = True


def build_program():
    nc = bacc.Bacc("TRN2", target_bir_lowering=False, debug=False)

    x_d = nc.dram_tensor("x", [S, D], BF16, kind="ExternalInput").ap()
    # g/d projections fp8 (x256); v projection bf16 (tanh path dominates error)
    wmix_d = nc.dram_tensor("w_mix", [2 * MD, 128, KD, 128], FP8, kind="ExternalInput").ap()
    wv_d = nc.dram_tensor("w_v", [MD, 128, KD, 128], BF16, kind="ExternalInput").ap()
    bmix_d = nc.dram_tensor("b_mix", [128, 3 * MD], F32, kind="ExternalInput").ap()
    wgu_d = nc.dram_tensor("w_gu", [MF2, 128, KD, 128], FP8, kind="ExternalInput").ap()
    wout_d = nc.dram_tensor("w_out", [MD, 128, MFO, 128], FP8, kind="ExternalInput").ap()
    ident_d = nc.dram_tensor("ident", [128, 128], BF16, kind="ExternalInput").ap()
    ones2_d = nc.dram_tensor("ones2", [128, 2, 16], FP8, kind="ExternalInput").ap()
    out_d = nc.dram_tensor("out", [S, D], F32, kind="ExternalOutput").ap()

    with tile.TileContext(nc) as tc, ExitStack() as top:
        # ---------- persistent tiles ----------
        cpool = top.enter_context(tc.tile_pool(name="consts", bufs=1))
        ident = cpool.tile([128, 128], BF16)
        nc.sync.dma_start(ident[:], ident_d[:])
        ones2 = cpool.tile([128, 2, 16], FP8)
        nc.sync.dma_start(ones2[:], ones2_d[:])
        ones_row = cpool.tile([1, 128], F32)
        nc.vector.memset(ones_row[:], 1.0)
        eps1 = cpool.tile([1, 1], F32)
        nc.vector.memset(eps1[:], EPS)
        bmix = cpool.tile([128, 3 * MD], F32)
        nc.sync.dma_start(bmix[:], bmix_d[:])

        # out1 (x + h) stays resident in SBUF across phase 1 -> 2b (bf16);
        # normalized out1 resident as fp8 for the FFN matmuls.
        res_pool = top.enter_context(tc.tile_pool(name="resid", bufs=1))
        o1all = res_pool.tile([128, KD, S], BF16)
        o1n = res_pool.tile([128, KD, S], FP8)

        carry_pool = top.enter_context(tc.tile_pool(name="carry", bufs=1))
        carry = carry_pool.tile([128, KD], F32)

        # norm helpers outlive phase 1 (the last two norm2 finishes are
        # emitted between the two FFN gate/up passes)
        p_sq = top.enter_context(tc.tile_pool(name="sqbufs", bufs=2))
        p_row = top.enter_context(tc.tile_pool(name="rows", bufs=1))
        p_bcS = top.enter_context(tc.tile_pool(name="bcS", bufs=1))
        ps_bc = top.enter_context(tc.tile_pool(name="bc_ps", bufs=2, space="PSUM"))

        # ---------- phase 1: mixer (software-pipelined over chunks) ----------
        with ExitStack() as ph1:
            wpool = ph1.enter_context(tc.tile_pool(name="wmix", bufs=1))
            wmix = wpool.tile([128, 2 * MD, KD, 128], FP8)
            wmix_dp = wmix_d.rearrange("m p k j -> p m k j")
            wv16 = wpool.tile([128, MD, KD, 128], BF16)
            wv_dp = wv_d.rearrange("m p k j -> p m k j")

            p_xT = ph1.enter_context(tc.tile_pool(name="xT", bufs=3))
            p_x16 = ph1.enter_context(tc.tile_pool(name="x16", bufs=2))
            p_x8 = ph1.enter_context(tc.tile_pool(name="x8", bufs=2))
            p_16 = ph1.enter_context(tc.tile_pool(name="f16bufs", bufs=2))
            p_scan = ph1.enter_context(tc.tile_pool(name="scanbufs", bufs=1))
            ps_mm = ph1.enter_context(tc.tile_pool(name="mm_ps", bufs=3, space="PSUM"))

            st_front = {}   # c -> (xT, rrow1)
            st_bc1 = {}     # c -> bc1 bf16 SBUF row-broadcast tile
            st_body = {}    # c -> (xs, a_t)  [in-place in sig_g / sig_d]
            st_back = {}    # c -> (out1_slice, rrow2)
            st_sq1 = {}

            def front_t(c, ksplit=1):
                """load x chunk transposed via the DMA xbar, squares on ACT.

                ksplit=2 (prologue chunks) loads d-halves into contiguous
                destination slices so the norm reduce starts on the first
                k-half while the second still streams in."""
                s0 = c * CH
                xT = p_xT.tile([128, KD, CH], BF16, tag="xT", name=f"xT{c}")
                sq = p_sq.tile([128, KD, CH], FP8, tag="sq", name=f"sq1_{c}")
                kh = KD // ksplit
                dh = D // ksplit
                for h in range(ksplit):
                    nc.sync.dma_start_transpose(
                        xT[:, h * kh:(h + 1) * kh, :],
                        x_d[s0:s0 + CH, h * dh:(h + 1) * dh])
                    nc.scalar.activation(sq[:, h * kh:(h + 1) * kh, :],
                                         xT[:, h * kh:(h + 1) * kh, :], AF.Square)
                st_front[c] = (xT, None)
                st_sq1[c] = sq

            def norm_rows(sq, label):
                """fp8-DoubleRow ones-reduce + sqrt + fast reciprocal."""
                ss = ps_bc.tile([1, CH], F32, tag="bc", name=f"ss{label}")
                for kp in range(NKP):
                    nc.tensor.matmul(ss[:], ones2[:, :, 0:1],
                                     sq[:, 2 * kp:2 * kp + 2, :],
                                     start=(kp == 0), stop=(kp == NKP - 1),
                                     perf_mode=PM)
                srow = p_row.tile([1, CH], F32, tag=f"srow{label[0]}", name=f"srow{label}")
                nc.scalar.activation(srow[:], ss[:], AF.Sqrt, bias=eps1[:], scale=1.0 / D)
                rrow = p_row.tile([1, CH], F32, tag=f"rrow{label[0]}", name=f"rrow{label}")
                nc.vector.reciprocal_approx_fast(rrow[:], srow[:])
                return rrow

            def bcast(rrow, tag, name):
                """PE f32 row-broadcast -> bf16 SBUF copy (DVE)."""
                bc = ps_bc.tile([128, CH], F32, tag="bc", name=f"bcp_{name}")
                nc.tensor.matmul(bc[:], ones_row[:], rrow[:])
                bcS = p_bcS.tile([128, CH], BF16, tag=tag, name=f"bcS_{name}")
                nc.vector.tensor_copy(bcS[:], bc[:])
                return bcS

            st_ss1 = {}

            def front_red(c):
                sq = st_sq1[c]
                ss = ps_bc.tile([1, CH], F32, tag="bc", name=f"ss1_{c}")
                for kp in range(NKP):
                    nc.tensor.matmul(ss[:], ones2[:, :, 0:1],
                                     sq[:, 2 * kp:2 * kp + 2, :],
                                     start=(kp == 0), stop=(kp == NKP - 1),
                                     perf_mode=PM)
                st_ss1[c] = ss

            def front_sqrt(c):
                ss = st_ss1[c]
                srow = p_row.tile([1, CH], F32, tag="srow1", name=f"srow1_{c}")
                nc.scalar.activation(srow[:], ss[:], AF.Sqrt, bias=eps1[:], scale=1.0 / D)
                rrow = p_row.tile([1, CH], F32, tag="rrow1", name=f"rrow1_{c}")
                nc.vector.reciprocal_approx_fast(rrow[:], srow[:])
                st_front[c] = (st_front[c][0], rrow)

            def bcast1(c):
                st_bc1[c] = bcast(st_front[c][1], "bc1", f"1_{c}")

            st_prep = {}

            def prep(c):
                """normalized input: bf16 (DVE) + fp8 cast (ACT), one chunk
                ahead of body so PE never waits on the normalize chain."""
                xT = st_front[c][0]
                bc1 = st_bc1[c]
                xnTb = p_x16.tile([128, KD, CH], BF16, tag="xnTb", name=f"xnTb{c}")
                for kt in range(KD):
                    nc.vector.tensor_tensor(xnTb[:, kt], xT[:, kt], bc1[:], OP.mult)
                st_prep[c] = (xnTb, None)

            def prep_cast(c):
                xnTb, _ = st_prep[c]
                xnT = p_x8.tile([128, KD, CH], FP8, tag="xnT", name=f"xnT{c}")
                nc.scalar.copy(xnT[:], xnTb[:])
                st_prep[c] = (xnTb, xnT)

            def _proj(c, dst, fn, slot, bcol):
                xnTb, xnT = st_prep[c]
                for half in range(MD // 2):
                    ps = ps_mm.tile([128, 2, CH], F32, tag="mm",
                                    name=f"mm{c}_{slot}_{half}")
                    for mi in range(2):
                        m = half * 2 + mi
                        if slot is None:
                            for kt in range(KD):
                                nc.tensor.matmul(ps[:, mi], wv16[:, m, kt, :],
                                                 xnTb[:, kt, :],
                                                 start=(kt == 0),
                                                 stop=(kt == KD - 1))
                        else:
                            for kp in range(NKP):
                                nc.tensor.matmul(ps[:, mi],
                                                 wmix[:, slot + m, 2 * kp:2 * kp + 2, :],
                                                 xnT[:, 2 * kp:2 * kp + 2, :],
                                                 start=(kp == 0), stop=(kp == NKP - 1),
                                                 perf_mode=PM)
                    for mi in range(2):
                        m = half * 2 + mi
                        nc.scalar.activation(dst[:, m], ps[:, mi], fn,
                                             bias=bmix[:, bcol + m:bcol + m + 1],
                                             scale=(1.0 if slot is None else 1.0 / WS))

            def body_v(c):
                tanh_v = p_16.tile([128, MD, CH], F16, tag="tanh_v", bufs=1, name=f"tv{c}")
                _proj(c, tanh_v, AF.Tanh, None, MD)
                st_body[c] = tanh_v

            def body_gd(c):
                tanh_v = st_body[c]
                sig_g = p_16.tile([128, MD, CH], F16, tag="sig_g", name=f"sg{c}")
                sig_d = p_16.tile([128, MD, CH], F16, tag="sig_d", name=f"sd{c}")
                _proj(c, sig_g, AF.Sigmoid, 0, 0)
                _proj(c, sig_d, AF.Sigmoid, MD, 2 * MD)
                st_body[c] = (sig_g, sig_d, tanh_v)

            def xs_at(c):
                # in-place: xs into sig_g, a_t into sig_d (DVE, after prep so
                # the next chunk's normalize never queues behind these)
                sig_g, sig_d, tanh_v = st_body[c]
                nc.vector.tensor_tensor(sig_g[:], sig_g[:], tanh_v[:], OP.mult)
                nc.vector.tensor_scalar(sig_d[:], sig_d[:], 0.998, 0.001, OP.mult, OP.add)
                st_body[c] = (sig_g, sig_d)

            def back_scan(c):
                """scan, residual into resident out1, squares (DVE+Pool only)."""
                xs, a_t = st_body[c]
                xT = st_front[c][0]
                s0 = c * CH
                hT = p_scan.tile([128, KD, CH], F16, tag="hT", name=f"hT{c}")
                for kt in range(KD):
                    init = 0.0 if c == 0 else carry[:, kt:kt + 1]
                    nc.vector.tensor_tensor_scan(hT[:, kt], a_t[:, kt], xs[:, kt],
                                                 init, OP.mult, OP.add)
                if c + 1 < NCH:
                    nc.vector.tensor_copy(carry[:], hT[:, :, CH - 1])
                out1 = o1all[:, :, s0:s0 + CH]
                nc.vector.tensor_tensor(out1, xT[:], hT[:], OP.add)
                sq2 = p_sq.tile([128, KD, CH], FP8, tag="sq", name=f"sq2_{c}")
                nc.gpsimd.tensor_tensor(sq2[:], out1, out1, OP.mult)
                st_back[c] = (out1, sq2)

            def back_fin(c):
                """norm2 reduce/broadcast (inputs ready); fp8 o1n on GpSimd."""
                s0 = c * CH
                out1, sq2 = st_back[c]
                rrow = norm_rows(sq2, f"2_{c}")
                bcS = bcast(rrow, "bc2", f"2_{c}")
                for kt in range(KD):
                    nc.gpsimd.tensor_tensor(o1n[:, kt, s0:s0 + CH],
                                            out1[:, kt], bcS[:], OP.mult)

            # pipelined emission: prep runs one chunk ahead of body (PE never
            # waits on the normalize chain); norm2 finish lags two chunks.
            front_t(0, ksplit=2)
            front_t(1, ksplit=2)
            front_t(2)
            for mt in range(MD):
                nc.sync.dma_start(wv16[:, mt], wv_dp[:, mt])
            for mt in range(2 * MD):
                nc.sync.dma_start(wmix[:, mt], wmix_dp[:, mt])
            front_red(0)
            front_sqrt(0)
            bcast1(0)
            prep(0)
            prep_cast(0)
            for c in range(NCH):
                if 3 <= c + 1 < NCH:
                    front_t(c + 1)
                if c >= 2:
                    back_fin(c - 2)
                body_v(c)
                if c + 1 < NCH:
                    front_red(c + 1)
                    front_sqrt(c + 1)
                    bcast1(c + 1)
                    prep(c + 1)
                body_gd(c)
                if c + 1 < NCH:
                    prep_cast(c + 1)
                xs_at(c)
                if c >= 1:
                    back_scan(c - 1)
            back_scan(NCH - 1)

        # ---------- phase 2: FFN ----------
        with ExitStack() as ph2:
            zpool = ph2.enter_context(tc.tile_pool(name="zbuf", bufs=1))
            z = zpool.tile([128, MFO, S], FP8)
            wo_pool = ph2.enter_context(tc.tile_pool(name="wout", bufs=1))
            wout = wo_pool.tile([128, MD, MFO, 128], FP8)
            wout_dp = wout_d.rearrange("m p k j -> p m k j")
            for mo in range(MD):
                nc.sync.dma_start(wout[:, mo], wout_dp[:, mo])

            # 2a: gate/up + z.  sc=3 deferred so PE need not wait on the
            # final o1n chunk while earlier chunks still have work.
            with ExitStack() as ph2a:
                p_wgu = ph2a.enter_context(tc.tile_pool(name="wgu", bufs=6))
                p_g16 = ph2a.enter_context(tc.tile_pool(name="g16", bufs=3))
                ps_gu = ph2a.enter_context(tc.tile_pool(name="gu_ps", bufs=6, space="PSUM"))
                for scs in ([0, 1], [2, 3]):
                    if scs[0] == 2:
                        back_fin(NCH - 2)
                        back_fin(NCH - 1)
                    for mg in range(MFO):
                        wg = p_wgu.tile([128, KD, 128], FP8, tag="wgu")
                        nc.sync.dma_start(wg[:], wgu_d[mg])
                        wu = p_wgu.tile([128, KD, 128], FP8, tag="wgu")
                        nc.sync.dma_start(wu[:], wgu_d[MFO + mg])
                        for sc in scs:
                            sl = slice(sc * CH, (sc + 1) * CH)
                            gps = ps_gu.tile([128, CH], F32, tag="gups")
                            for kp in range(NKP):
                                nc.tensor.matmul(gps[:], wg[:, 2 * kp:2 * kp + 2, :],
                                                 o1n[:, 2 * kp:2 * kp + 2, sl],
                                                 start=(kp == 0), stop=(kp == NKP - 1),
                                                 perf_mode=PM)
                            ups = ps_gu.tile([128, CH], F32, tag="gups")
                            for kp in range(NKP):
                                nc.tensor.matmul(ups[:], wu[:, 2 * kp:2 * kp + 2, :],
                                                 o1n[:, 2 * kp:2 * kp + 2, sl],
                                                 start=(kp == 0), stop=(kp == NKP - 1),
                                                 perf_mode=PM)
                            gate = p_g16.tile([128, CH], F16, tag="gate")
                            if SILU_ON_ACT:
                                nc.scalar.activation(gate[:], gps[:], AF.Silu,
                                                     scale=1.0 / WS)
                            else:
                                sig = p_g16.tile([128, CH], F16, tag="sig")
                                nc.scalar.activation(sig[:], gps[:], AF.Sigmoid,
                                                     scale=1.0 / WS)
                                nc.vector.scalar_tensor_tensor(gate[:], gps[:], 1.0 / WS,
                                                               sig[:], OP.mult, OP.mult)
                            nc.vector.scalar_tensor_tensor(z[:, mg, sl], ups[:], ZS / WS,
                                                           gate[:], OP.mult, OP.mult)

            # 2b: W_out + residual (from resident out1) + transpose out
            with ExitStack() as ph2b:
                p_oT = ph2b.enter_context(tc.tile_pool(name="outT", bufs=MD + 1))
                p_onat = ph2b.enter_context(tc.tile_pool(name="onat", bufs=3))
                ps_y = ph2b.enter_context(tc.tile_pool(name="y_ps", bufs=2, space="PSUM"))
                ps_t2 = ph2b.enter_context(tc.tile_pool(name="t2_ps", bufs=2, space="PSUM"))
                for sc in range(NCH):
                    sl = slice(sc * CH, (sc + 1) * CH)
                    outTs = []
                    for mo in range(MD):
                        yps = ps_y.tile([128, CH], F32, tag="yps")
                        for op_ in range(NOP):
                            nc.tensor.matmul(yps[:], wout[:, mo, 2 * op_:2 * op_ + 2, :],
                                             z[:, 2 * op_:2 * op_ + 2, sl],
                                             start=(op_ == 0), stop=(op_ == NOP - 1),
                                             perf_mode=PM)
                        oT = p_oT.tile([128, CH], BF16, tag="oT")
                        nc.vector.scalar_tensor_tensor(oT[:], yps[:], 1.0 / (WS * ZS),
                                                       o1all[:, mo, sl], OP.mult, OP.add)
                        outTs.append(oT)
                    for q in range(NST):
                        onat = p_onat.tile([128, D], F32, tag="onat")
                        for h in range(2):
                            t2 = ps_t2.tile([128, 512], BF16, tag="t2")
                            for j in range(4):
                                nc.tensor.transpose(
                                    t2[:, j * 128:(j + 1) * 128],
                                    outTs[4 * h + j][:, q * 128:(q + 1) * 128],
                                    ident[:])
                            nc.scalar.copy(onat[:, h * 512:(h + 1) * 512], t2[:])
                        srow0 = sc * CH + q * 128
                        nc.sync.dma_start(out_d[srow0:srow0 + 128, :], onat[:])

    nc.compile()
    return nc


_NC = None


def _get_nc():
    global _NC
    if _NC is None:
        _NC = build_program()
    return _NC


def _q8(a):
    return np.clip(a, -240.0, 240.0).astype(ml_dtypes.float8_e4m3)


def _prep_weights(inputs):
    w1 = np.asarray(inputs["rms_mix_w"], np.float32)
    w2 = np.asarray(inputs["rms_ffn_w"], np.float32)
    Wg = np.asarray(inputs["Wg"], np.float32) * w1[None, :]
    Wv = np.asarray(inputs["Wv"], np.float32) * w1[None, :]
    Wd = np.asarray(inputs["Wd"], np.float32) * w1[None, :]
    Wcat = np.concatenate([Wg, Wd], axis=0) * WS            # [2D, D] fp8 x256
    w_mix = _q8(np.ascontiguousarray(
        Wcat.T.reshape(KD, 128, 2 * MD, 128).transpose(2, 1, 0, 3)))
    w_v = np.ascontiguousarray(
        Wv.T.reshape(KD, 128, MD, 128).transpose(2, 1, 0, 3)).astype(ml_dtypes.bfloat16)
    bcat = np.concatenate([np.asarray(inputs["bg"], np.float32),
                           np.asarray(inputs["bv"], np.float32),
                           np.asarray(inputs["bd"], np.float32)])
    b_mix = np.ascontiguousarray(bcat.reshape(3 * MD, 128).T).astype(np.float32)
    Wgate = np.asarray(inputs["W_gate"], np.float32) * w2[None, :]
    Wup = np.asarray(inputs["W_up"], np.float32) * w2[None, :]
    Wcat2 = np.concatenate([Wgate, Wup], axis=0) * WS       # [2F, D]
    w_gu = _q8(np.ascontiguousarray(
        Wcat2.T.reshape(KD, 128, MF2, 128).transpose(2, 1, 0, 3)))
    WoT = np.asarray(inputs["W_out"], np.float32).T * WS    # [F, D]
    w_out = _q8(np.ascontiguousarray(
        WoT.reshape(MFO, 128, MD, 128).transpose(2, 1, 0, 3)))
    return {
        "w_mix": w_mix, "w_v": w_v, "b_mix": b_mix, "w_gu": w_gu, "w_out": w_out,
        "ident": np.eye(128).astype(ml_dtypes.bfloat16),
        "ones2": np.ones((128, 2, 16)).astype(ml_dtypes.float8_e4m3),
    }


def run(inputs, trace=False, **kw):
    x = np.asarray(inputs["x"], np.float32)
    shared = _prep_weights(inputs)
    in_maps = [
        dict(shared, x=np.ascontiguousarray(x[b]).astype(ml_dtypes.bfloat16))
        for b in range(B)
    ]
    res = run_bass_kernel_spmd(_get_nc(), in_maps, list(range(B)), trace=trace, **kw)
    out = np.stack([np.asarray(res.results[b]["out"], np.float32) for b in range(B)])
    return out, res


def kernel(**inputs) -> np.ndarray:
    out, _ = run(inputs)
    return out


# revision 46
# speedup vs baseline: 1.0170x; 1.0061x over previous
"""MinGRU block kernel for Trainium2 (Bass/Tile), SPMD over 8 NeuronCores.

Problem: B=8, S=2048, D=1024, F=3072 (nn_MinGRUBlock).
Sharding: data-parallel over batch (one batch row per core); weights replicated.

fp8 edition: the g/d projections, FFN gate/up, and W_out matmuls run as
float8_e4m3 DoubleRow matmuls (2 k-planes per pass); the v projection stays
bf16 (the tanh path dominates the error budget). fp8 weights are pre-scaled
x256 on the host so |w| clears the fp8 subnormal range; the 1/256 is folded
into the ACT activation `scale` when reading PSUM.

Per-core dataflow (compute in "T layout": feature on partitions, time on free):
  phase 1 (mixer, s-chunks of 512):
    x loaded in bf16, PE-transposed -> xT [d,s] bf16
    rms row-sums: squares on GpSimd (fp8 out), PE fp8-DoubleRow ones-reduce,
    sqrt (ACT) + reciprocal_approx_fast (DVE), PE f32 broadcast -> bf16 SBUF
    xnTb = xT * r (DVE bf16, 2x mode); xnT fp8 cast on ACT
    v: bf16 matmuls; g/d: fp8 DoubleRow matmuls; activations on ACT
    xs/a_t computed in place (DVE); h_t = a_t*h + x_t via tensor_tensor_scan
    out1 = x + h (bf16) into a resident [128, KD, S] tile (no DRAM bounce);
    o1n = fp8(out1*r2) on GpSimd, resident
  phase 2 (FFN): wgu streamed, wout resident;
    gate = silu(gps/256) on ACT (Sigmoid+mults when SILU_ON_ACT=False, for
    CoreSim which lacks Silu); z = (ups*16/256)*gate -> fp8 (DVE
    scalar_tensor_tensor); W_out fp8 DoubleRow; residual = (yps/4096 + out1)
    via one DVE scalar_tensor_tensor reading the resident out1; bf16
    PE-transpose back; f32 out.
"""

import os
import sys
from contextlib import ExitStack

import numpy as np
import ml_dtypes

for _p in ("/opt/trn_rl_repo", "/root/.axon_site/_ro/trn_rl_repo"):
    if os.path.isdir(_p) and _p not in sys.path:
        sys.path.insert(0, _p)

import concourse.bass as bass
import concourse.tile as tile
from concourse import bacc, mybir
from concourse.bass_utils import run_bass_kernel_spmd

F32 = mybir.dt.float32
F16 = mybir.dt.float16
BF16 = mybir.dt.bfloat16
FP8 = mybir.dt.float8e4
AF = mybir.ActivationFunctionType
OP = mybir.AluOpType
PM = mybir.MatmulPerfMode.DoubleRow

B, S, D, F = 8, 2048, 1024, 3072
EPS = 1e-6
KD = D // 128          # 8 d-ptiles
NKP = KD // 2          # 4 d-pair tiles
MF2 = 2 * F // 128     # 48 f-ptiles (gate|up)
MFO = F // 128         # 24 f-ptiles
NOP = MFO // 2         # 12 f-pair tiles
MD = D // 128          # 8 d-ptiles (output)

CH = 512               # s-chunk (both phases)
NCH = S // CH          # 4
NST = CH // 128        # 4 s-tiles per chunk

WS = 256.0             # weight fp8 pre-scale
ZS = 16.0              # z fp8 pre-scale

# Hardware ACT has Silu in its function tables; CoreSim does not implement
# it. sim_check.py flips this off to validate structure/numerics in sim.
SILU_ON_ACT = True


def build_program():
    nc = bacc.Bacc("TRN2", target_bir_lowering=False, debug=False)

    x_d = nc.dram_tensor("x", [S, D], BF16, kind="ExternalInput").ap()
    # g/d projections fp8 (x256); v projection bf16 (tanh path dominates error)
    wmix_d = nc.dram_tensor("w_mix", [2 * MD, 128, KD, 128], FP8, kind="ExternalInput").ap()
    wv_d = nc.dram_tensor("w_v", [MD, 128, KD, 128], BF16, kind="ExternalInput").ap()
    bmix_d = nc.dram_tensor("b_mix", [128, 3 * MD], F32, kind="ExternalInput").ap()
    wgu_d = nc.dram_tensor("w_gu", [MF2, 128, KD, 128], FP8, kind="ExternalInput").ap()
    wout_d = nc.dram_tensor("w_out", [MD, 128, MFO, 128], FP8, kind="ExternalInput").ap()
    ident_d = nc.dram_tensor("ident", [128, 128], BF16, kind="ExternalInput").ap()
    ones2_d = nc.dram_tensor("ones2", [128, 2, 16], FP8, kind="ExternalInput").ap()
    out_d = nc.dram_tensor("out", [S, D], F32, kind="ExternalOutput").ap()

    with tile.TileContext(nc) as tc, ExitStack() as top:
        # ---------- persistent tiles ----------
        cpool = top.enter_context(tc.tile_pool(name="consts", bufs=1))
        ident = cpool.tile([128, 128], BF16)
        nc.sync.dma_start(ident[:], ident_d[:])
        ones2 = cpool.tile([128, 2, 16], FP8)
        nc.sync.dma_start(ones2[:], ones2_d[:])
        ones_row = cpool.tile([1, 128], F32)
        nc.vector.memset(ones_row[:], 1.0)
        eps1 = cpool.tile([1, 1], F32)
        nc.vector.memset(eps1[:], EPS)
        bmix = cpool.tile([128, 3 * MD], F32)
        nc.sync.dma_start(bmix[:], bmix_d[:])

        # out1 (x + h) stays resident in SBUF across phase 1 -> 2b (bf16);
        # normalized out1 resident as fp8 for the FFN matmuls.
        res_pool = top.enter_context(tc.tile_pool(name="resid", bufs=1))
        o1all = res_pool.tile([128, KD, S], BF16)
        o1n = res_pool.tile([128, KD, S], FP8)

        carry_pool = top.enter_context(tc.tile_pool(name="carry", bufs=1))
        carry = carry_pool.tile([128, KD], F32)

        # norm helpers outlive phase 1 (the last two norm2 finishes are
        # emitted between the two FFN gate/up passes)
        p_sq = top.enter_context(tc.tile_pool(name="sqbufs", bufs=2))
        p_row = top.enter_context(tc.tile_pool(name="rows", bufs=1))
        p_bcS = top.enter_context(tc.tile_pool(name="bcS", bufs=1))
        ps_bc = top.enter_context(tc.tile_pool(name="bc_ps", bufs=2, space="PSUM"))

        # ---------- phase 1: mixer (software-pipelined over chunks) ----------
        with ExitStack() as ph1:
            wpool = ph1.enter_context(tc.tile_pool(name="wmix", bufs=1))
            wmix = wpool.tile([128, 2 * MD, KD, 128], FP8)
            wmix_dp = wmix_d.rearrange("m p k j -> p m k j")
            wv16 = wpool.tile([128, MD, KD, 128], BF16)
            wv_dp = wv_d.rearrange("m p k j -> p m k j")

            p_xT = ph1.enter_context(tc.tile_pool(name="xT", bufs=3))
            p_x16 = ph1.enter_context(tc.tile_pool(name="x16", bufs=2))
            p_x8 = ph1.enter_context(tc.tile_pool(name="x8", bufs=2))
            p_16 = ph1.enter_context(tc.tile_pool(name="f16bufs", bufs=2))
            p_scan = ph1.enter_context(tc.tile_pool(name="scanbufs", bufs=1))
            ps_mm = ph1.enter_context(tc.tile_pool(name="mm_ps", bufs=3, space="PSUM"))

            st_front = {}   # c -> (xT, rrow1)
            st_bc1 = {}     # c -> bc1 bf16 SBUF row-broadcast tile
            st_body = {}    # c -> (xs, a_t)  [in-place in sig_g / sig_d]
            st_back = {}    # c -> (out1_slice, rrow2)
            st_sq1 = {}

            def front_t(c):
                """load x chunk transposed via the DMA xbar, squares on GpSimd."""
                s0 = c * CH
                xT = p_xT.tile([128, KD, CH], BF16, tag="xT", name=f"xT{c}")
                nc.sync.dma_start_transpose(xT[:], x_d[s0:s0 + CH, :])
                sq = p_sq.tile([128, KD, CH], FP8, tag="sq", name=f"sq1_{c}")
                nc.scalar.activation(sq[:], xT[:], AF.Square)
                st_front[c] = (xT, None)
                st_sq1[c] = sq

            def norm_rows(sq, label):
                """fp8-DoubleRow ones-reduce + sqrt + fast reciprocal."""
                ss = ps_bc.tile([1, CH], F32, tag="bc", name=f"ss{label}")
                for kp in range(NKP):
                    nc.tensor.matmul(ss[:], ones2[:, :, 0:1],
                                     sq[:, 2 * kp:2 * kp + 2, :],
                                     start=(kp == 0), stop=(kp == NKP - 1),
                                     perf_mode=PM)
                srow = p_row.tile([1, CH], F32, tag=f"srow{label[0]}", name=f"srow{label}")
                nc.scalar.activation(srow[:], ss[:], AF.Sqrt, bias=eps1[:], scale=1.0 / D)
                rrow = p_row.tile([1, CH], F32, tag=f"rrow{label[0]}", name=f"rrow{label}")
                nc.vector.reciprocal_approx_fast(rrow[:], srow[:])
                return rrow

            def bcast(rrow, tag, name):
                """PE f32 row-broadcast -> bf16 SBUF copy (DVE)."""
                bc = ps_bc.tile([128, CH], F32, tag="bc", name=f"bcp_{name}")
                nc.tensor.matmul(bc[:], ones_row[:], rrow[:])
                bcS = p_bcS.tile([128, CH], BF16, tag=tag, name=f"bcS_{name}")
                nc.vector.tensor_copy(bcS[:], bc[:])
                return bcS

            st_ss1 = {}

            def front_red(c):
                sq = st_sq1[c]
                ss = ps_bc.tile([1, CH], F32, tag="bc", name=f"ss1_{c}")
                for kp in range(NKP):
                    nc.tensor.matmul(ss[:], ones2[:, :, 0:1],
                                     sq[:, 2 * kp:2 * kp + 2, :],
                                     start=(kp == 0), stop=(kp == NKP - 1),
                                     perf_mode=PM)
                st_ss1[c] = ss

            def front_sqrt(c):
                ss = st_ss1[c]
                srow = p_row.tile([1, CH], F32, tag="srow1", name=f"srow1_{c}")
                nc.scalar.activation(srow[:], ss[:], AF.Sqrt, bias=eps1[:], scale=1.0 / D)
                rrow = p_row.tile([1, CH], F32, tag="rrow1", name=f"rrow1_{c}")
                nc.vector.reciprocal_approx_fast(rrow[:], srow[:])
                st_front[c] = (st_front[c][0], rrow)

            def bcast1(c):
                st_bc1[c] = bcast(st_front[c][1], "bc1", f"1_{c}")

            st_prep = {}

            def prep(c):
                """normalized input: bf16 (DVE) + fp8 cast (ACT), one chunk
                ahead of body so PE never waits on the normalize chain."""
                xT = st_front[c][0]
                bc1 = st_bc1[c]
                xnTb = p_x16.tile([128, KD, CH], BF16, tag="xnTb", name=f"xnTb{c}")
                for kt in range(KD):
                    nc.vector.tensor_tensor(xnTb[:, kt], xT[:, kt], bc1[:], OP.mult)
                st_prep[c] = (xnTb, None)

            def prep_cast(c):
                xnTb, _ = st_prep[c]
                xnT = p_x8.tile([128, KD, CH], FP8, tag="xnT", name=f"xnT{c}")
                nc.scalar.copy(xnT[:], xnTb[:])
                st_prep[c] = (xnTb, xnT)

            def _proj(c, dst, fn, slot, bcol):
                xnTb, xnT = st_prep[c]
                for half in range(MD // 2):
                    ps = ps_mm.tile([128, 2, CH], F32, tag="mm",
                                    name=f"mm{c}_{slot}_{half}")
                    for mi in range(2):
                        m = half * 2 + mi
                        if slot is None:
                            for kt in range(KD):
                                nc.tensor.matmul(ps[:, mi], wv16[:, m, kt, :],
                                                 xnTb[:, kt, :],
                                                 start=(kt == 0),
                                                 stop=(kt == KD - 1))
                        else:
                            for kp in range(NKP):
                                nc.tensor.matmul(ps[:, mi],
                                                 wmix[:, slot + m, 2 * kp:2 * kp + 2, :],
                                                 xnT[:, 2 * kp:2 * kp + 2, :],
                                                 start=(kp == 0), stop=(kp == NKP - 1),
                                                 perf_mode=PM)
                    for mi in range(2):
                        m = half * 2 + mi
                        nc.scalar.activation(dst[:, m], ps[:, mi], fn,
                                             bias=bmix[:, bcol + m:bcol + m + 1],
                                             scale=(1.0 if slot is None else 1.0 / WS))

            def body_v(c):
                tanh_v = p_16.tile([128, MD, CH], F16, tag="tanh_v", bufs=1, name=f"tv{c}")
                _proj(c, tanh_v, AF.Tanh, None, MD)
                st_body[c] = tanh_v

            def body_gd(c):
                tanh_v = st_body[c]
                sig_g = p_16.tile([128, MD, CH], F16, tag="sig_g", name=f"sg{c}")
                sig_d = p_16.tile([128, MD, CH], F16, tag="sig_d", name=f"sd{c}")
                _proj(c, sig_g, AF.Sigmoid, 0, 0)
                _proj(c, sig_d, AF.Sigmoid, MD, 2 * MD)
                st_body[c] = (sig_g, sig_d, tanh_v)

            def xs_at(c):
                # in-place: xs into sig_g, a_t into sig_d (DVE, after prep so
                # the next chunk's normalize never queues behind these)
                sig_g, sig_d, tanh_v = st_body[c]
                nc.vector.tensor_tensor(sig_g[:], sig_g[:], tanh_v[:], OP.mult)
                nc.vector.tensor_scalar(sig_d[:], sig_d[:], 0.998, 0.001, OP.mult, OP.add)
                st_body[c] = (sig_g, sig_d)

            def back_scan(c):
                """scan, residual into resident out1, squares (DVE+Pool only)."""
                xs, a_t = st_body[c]
                xT = st_front[c][0]
                s0 = c * CH
                hT = p_scan.tile([128, KD, CH], F16, tag="hT", name=f"hT{c}")
                for kt in range(KD):
                    init = 0.0 if c == 0 else carry[:, kt:kt + 1]
                    nc.vector.tensor_tensor_scan(hT[:, kt], a_t[:, kt], xs[:, kt],
                                                 init, OP.mult, OP.add)
                if c + 1 < NCH:
                    nc.vector.tensor_copy(carry[:], hT[:, :, CH - 1])
                out1 = o1all[:, :, s0:s0 + CH]
                nc.vector.tensor_tensor(out1, xT[:], hT[:], OP.add)
                sq2 = p_sq.tile([128, KD, CH], FP8, tag="sq", name=f"sq2_{c}")
                nc.gpsimd.tensor_tensor(sq2[:], out1, out1, OP.mult)
                st_back[c] = (out1, sq2)

            def back_fin(c):
                """norm2 reduce/broadcast (inputs ready); fp8 o1n on GpSimd."""
                s0 = c * CH
                out1, sq2 = st_back[c]
                rrow = norm_rows(sq2, f"2_{c}")
                bcS = bcast(rrow, "bc2", f"2_{c}")
                for kt in range(KD):
                    nc.gpsimd.tensor_tensor(o1n[:, kt, s0:s0 + CH],
                                            out1[:, kt], bcS[:], OP.mult)

            # pipelined emission: prep runs one chunk ahead of body (PE never
            # waits on the normalize chain); norm2 finish lags two chunks.
            front_t(0)
            front_t(1)
            front_t(2)
            for mt in range(MD):
                nc.sync.dma_start(wv16[:, mt], wv_dp[:, mt])
            for mt in range(2 * MD):
                nc.sync.dma_start(wmix[:, mt], wmix_dp[:, mt])
            front_red(0)
            front_sqrt(0)
            bcast1(0)
            prep(0)
            prep_cast(0)
            for c in range(NCH):
                if 3 <= c + 1 < NCH:
                    front_t(c + 1)
                if c >= 2:
                    back_fin(c - 2)
                body_v(c)
                if c + 1 < NCH:
                    front_red(c + 1)
                    front_sqrt(c + 1)
                    bcast1(c + 1)
                    prep(c + 1)
                body_gd(c)
                if c + 1 < NCH:
                    prep_cast(c + 1)
                xs_at(c)
                if c >= 1:
                    back_scan(c - 1)
            back_scan(NCH - 1)

        # ---------- phase 2: FFN ----------
        with ExitStack() as ph2:
            zpool = ph2.enter_context(tc.tile_pool(name="zbuf", bufs=1))
            z = zpool.tile([128, MFO, S], FP8)
            wo_pool = ph2.enter_context(tc.tile_pool(name="wout", bufs=1))
            wout = wo_pool.tile([128, MD, MFO, 128], FP8)
            wout_dp = wout_d.rearrange("m p k j -> p m k j")
            for mo in range(MD):
                nc.sync.dma_start(wout[:, mo], wout_dp[:, mo])

            # 2a: gate/up + z.  sc=3 deferred so PE need not wait on the
            # final o1n chunk while earlier chunks still have work.
            with ExitStack() as ph2a:
                p_wgu = ph2a.enter_context(tc.tile_pool(name="wgu", bufs=6))
                p_g16 = ph2a.enter_context(tc.tile_pool(name="g16", bufs=3))
                ps_gu = ph2a.enter_context(tc.tile_pool(name="gu_ps", bufs=4, space="PSUM"))
                for scs in ([0, 1], [2, 3]):
                    if scs[0] == 2:
                        back_fin(NCH - 2)
                        back_fin(NCH - 1)
                    for mg in range(MFO):
                        wg = p_wgu.tile([128, KD, 128], FP8, tag="wgu")
                        nc.sync.dma_start(wg[:], wgu_d[mg])
                        wu = p_wgu.tile([128, KD, 128], FP8, tag="wgu")
                        nc.sync.dma_start(wu[:], wgu_d[MFO + mg])
                        for sc in scs:
                            sl = slice(sc * CH, (sc + 1) * CH)
                            gps = ps_gu.tile([128, CH], F32, tag="gups")
                            for kp in range(NKP):
                                nc.tensor.matmul(gps[:], wg[:, 2 * kp:2 * kp + 2, :],
                                                 o1n[:, 2 * kp:2 * kp + 2, sl],
                                                 start=(kp == 0), stop=(kp == NKP - 1),
                                                 perf_mode=PM)
                            ups = ps_gu.tile([128, CH], F32, tag="gups")
                            for kp in range(NKP):
                                nc.tensor.matmul(ups[:], wu[:, 2 * kp:2 * kp + 2, :],
                                                 o1n[:, 2 * kp:2 * kp + 2, sl],
                                                 start=(kp == 0), stop=(kp == NKP - 1),
                                                 perf_mode=PM)
                            gate = p_g16.tile([128, CH], F16, tag="gate")
                            if SILU_ON_ACT:
                                nc.scalar.activation(gate[:], gps[:], AF.Silu,
                                                     scale=1.0 / WS)
                            else:
                                sig = p_g16.tile([128, CH], F16, tag="sig")
                                nc.scalar.activation(sig[:], gps[:], AF.Sigmoid,
                                                     scale=1.0 / WS)
                                nc.vector.scalar_tensor_tensor(gate[:], gps[:], 1.0 / WS,
                                                               sig[:], OP.mult, OP.mult)
                            nc.vector.scalar_tensor_tensor(z[:, mg, sl], ups[:], ZS / WS,
                                                           gate[:], OP.mult, OP.mult)

            # 2b: W_out + residual (from resident out1) + transpose out
            with ExitStack() as ph2b:
                p_oT = ph2b.enter_context(tc.tile_pool(name="outT", bufs=MD + 1))
                p_onat = ph2b.enter_context(tc.tile_pool(name="onat", bufs=3))
                ps_y = ph2b.enter_context(tc.tile_pool(name="y_ps", bufs=2, space="PSUM"))
                ps_t2 = ph2b.enter_context(tc.tile_pool(name="t2_ps", bufs=2, space="PSUM"))
                for sc in range(NCH):
                    sl = slice(sc * CH, (sc + 1) * CH)
                    outTs = []
                    for mo in range(MD):
                        yps = ps_y.tile([128, CH], F32, tag="yps")
                        for op_ in range(NOP):
                            nc.tensor.matmul(yps[:], wout[:, mo, 2 * op_:2 * op_ + 2, :],
                                             z[:, 2 * op_:2 * op_ + 2, sl],
                                             start=(op_ == 0), stop=(op_ == NOP - 1),
                                             perf_mode=PM)
                        oT = p_oT.tile([128, CH], BF16, tag="oT")
                        nc.vector.scalar_tensor_tensor(oT[:], yps[:], 1.0 / (WS * ZS),
                                                       o1all[:, mo, sl], OP.mult, OP.add)
                        outTs.append(oT)
                    for q in range(NST):
                        onat = p_onat.tile([128, D], F32, tag="onat")
                        for h in range(2):
                            t2 = ps_t2.tile([128, 512], BF16, tag="t2")
                            for j in range(4):
                                nc.tensor.transpose(
                                    t2[:, j * 128:(j + 1) * 128],
                                    outTs[4 * h + j][:, q * 128:(q + 1) * 128],
                                    ident[:])
                            nc.scalar.copy(onat[:, h * 512:(h + 1) * 512], t2[:])
                        srow0 = sc * CH + q * 128
                        nc.sync.dma_start(out_d[srow0:srow0 + 128, :], onat[:])

    nc.compile()
    return nc


_NC = None


def _get_nc():
    global _NC
    if _NC is None:
        _NC = build_program()
    return _NC


def _q8(a):
    return np.clip(a, -240.0, 240.0).astype(ml_dtypes.float8_e4m3)


def _prep_weights(inputs):
    w1 = np.asarray(inputs["rms_mix_w"], np.float32)
    w2 = np.asarray(inputs["rms_ffn_w"], np.float32)
    Wg = np.asarray(inputs["Wg"], np.float32) * w1[None, :]
    Wv = np.asarray(inputs["Wv"], np.float32) * w1[None, :]
    Wd = np.asarray(inputs["Wd"], np.float32) * w1[None, :]
    Wcat = np.concatenate([Wg, Wd], axis=0) * WS            # [2D, D] fp8 x256
    w_mix = _q8(np.ascontiguousarray(
        Wcat.T.reshape(KD, 128, 2 * MD, 128).transpose(2, 1, 0, 3)))
    w_v = np.ascontiguousarray(
        Wv.T.reshape(KD, 128, MD, 128).transpose(2, 1, 0, 3)).astype(ml_dtypes.bfloat16)
    bcat = np.concatenate([np.asarray(inputs["bg"], np.float32),
                           np.asarray(inputs["bv"], np.float32),
                           np.asarray(inputs["bd"], np.float32)])
    b_mix = np.ascontiguousarray(bcat.reshape(3 * MD, 128).T).astype(np.float32)
    Wgate = np.asarray(inputs["W_gate"], np.float32) * w2[None, :]
    Wup = np.asarray(inputs["W_up"], np.float32) * w2[None, :]
    Wcat2 = np.concatenate([Wgate, Wup], axis=0) * WS       # [2F, D]
    w_gu = _q8(np.ascontiguousarray(
        Wcat2.T.reshape(KD, 128, MF2, 128).transpose(2, 1, 0, 3)))
    WoT = np.asarray(inputs["W_out"], np.float32).T * WS    # [F, D]
    w_out = _q8(np.ascontiguousarray(
        WoT.reshape(MFO, 128, MD, 128).transpose(2, 1, 0, 3)))
    return {
        "w_mix": w_mix, "w_v": w_v, "b_mix": b_mix, "w_gu": w_gu, "w_out": w_out,
        "ident": np.eye(128).astype(ml_dtypes.bfloat16),
        "ones2": np.ones((128, 2, 16)).astype(ml_dtypes.float8_e4m3),
    }


def run(inputs, trace=False, **kw):
    x = np.asarray(inputs["x"], np.float32)
    shared = _prep_weights(inputs)
    in_maps = [
        dict(shared, x=np.ascontiguousarray(x[b]).astype(ml_dtypes.bfloat16))
        for b in range(B)
    ]
    res = run_bass_kernel_spmd(_get_nc(), in_maps, list(range(B)), trace=trace, **kw)
    out = np.stack([np.asarray(res.results[b]["out"], np.float32) for b in range(B)])
    return out, res


def kernel(**inputs) -> np.ndarray:
    out, _ = run(inputs)
    return out


# revision 47
# speedup vs baseline: 1.0244x; 1.0072x over previous
"""MinGRU block kernel for Trainium2 (Bass/Tile), SPMD over 8 NeuronCores.

Problem: B=8, S=2048, D=1024, F=3072 (nn_MinGRUBlock).
Sharding: data-parallel over batch (one batch row per core); weights replicated.

fp8 edition: the g/d projections, FFN gate/up, and W_out matmuls run as
float8_e4m3 DoubleRow matmuls (2 k-planes per pass); the v projection stays
bf16 (the tanh path dominates the error budget). fp8 weights are pre-scaled
x256 on the host so |w| clears the fp8 subnormal range; the 1/256 is folded
into the ACT activation `scale` when reading PSUM.

Per-core dataflow (compute in "T layout": feature on partitions, time on free):
  phase 1 (mixer, s-chunks of 512):
    x loaded in bf16, PE-transposed -> xT [d,s] bf16
    rms row-sums: squares on GpSimd (fp8 out), PE fp8-DoubleRow ones-reduce,
    sqrt (ACT) + reciprocal_approx_fast (DVE), PE f32 broadcast -> bf16 SBUF
    xnTb = xT * r (DVE bf16, 2x mode); xnT fp8 cast on ACT
    v: bf16 matmuls; g/d: fp8 DoubleRow matmuls; activations on ACT
    xs/a_t computed in place (DVE); h_t = a_t*h + x_t via tensor_tensor_scan
    out1 = x + h (bf16) into a resident [128, KD, S] tile (no DRAM bounce);
    o1n = fp8(out1*r2) on GpSimd, resident
  phase 2 (FFN): wgu streamed, wout resident;
    gate = silu(gps/256) on ACT (Sigmoid+mults when SILU_ON_ACT=False, for
    CoreSim which lacks Silu); z = (ups*16/256)*gate -> fp8 (DVE
    scalar_tensor_tensor); W_out fp8 DoubleRow; residual = (yps/4096 + out1)
    via one DVE scalar_tensor_tensor reading the resident out1; bf16
    PE-transpose back; f32 out.
"""

import os
import sys
from contextlib import ExitStack

import numpy as np
import ml_dtypes

for _p in ("/opt/trn_rl_repo", "/root/.axon_site/_ro/trn_rl_repo"):
    if os.path.isdir(_p) and _p not in sys.path:
        sys.path.insert(0, _p)

import concourse.bass as bass
import concourse.tile as tile
from concourse import bacc, mybir
from concourse.bass_utils import run_bass_kernel_spmd

F32 = mybir.dt.float32
F16 = mybir.dt.float16
BF16 = mybir.dt.bfloat16
FP8 = mybir.dt.float8e4
AF = mybir.ActivationFunctionType
OP = mybir.AluOpType
PM = mybir.MatmulPerfMode.DoubleRow

B, S, D, F = 8, 2048, 1024, 3072
EPS = 1e-6
KD = D // 128          # 8 d-ptiles
NKP = KD // 2          # 4 d-pair tiles
MF2 = 2 * F // 128     # 48 f-ptiles (gate|up)
MFO = F // 128         # 24 f-ptiles
NOP = MFO // 2         # 12 f-pair tiles
MD = D // 128          # 8 d-ptiles (output)

CH = 512               # s-chunk (both phases)
NCH = S // CH          # 4
NST = CH // 128        # 4 s-tiles per chunk

WS = 256.0             # weight fp8 pre-scale
ZS = 16.0              # z fp8 pre-scale

# Hardware ACT has Silu in its function tables; CoreSim does not implement
# it. sim_check.py flips this off to validate structure/numerics in sim.
SILU_ON_ACT = True


def build_program():
    nc = bacc.Bacc("TRN2", target_bir_lowering=False, debug=False)

    x_d = nc.dram_tensor("x", [S, D], BF16, kind="ExternalInput").ap()
    # g/d projections fp8 (x256); v projection bf16 (tanh path dominates error)
    wmix_d = nc.dram_tensor("w_mix", [2 * MD, 128, KD, 128], FP8, kind="ExternalInput").ap()
    wv_d = nc.dram_tensor("w_v", [MD, 128, KD, 128], BF16, kind="ExternalInput").ap()
    bmix_d = nc.dram_tensor("b_mix", [128, 3 * MD], F32, kind="ExternalInput").ap()
    wgu_d = nc.dram_tensor("w_gu", [MF2, 128, KD, 128], FP8, kind="ExternalInput").ap()
    wout_d = nc.dram_tensor("w_out", [MD, 128, MFO, 128], FP8, kind="ExternalInput").ap()
    ident_d = nc.dram_tensor("ident", [128, 128], BF16, kind="ExternalInput").ap()
    ones2_d = nc.dram_tensor("ones2", [128, 2, 16], FP8, kind="ExternalInput").ap()
    out_d = nc.dram_tensor("out", [S, D], F32, kind="ExternalOutput").ap()

    with tile.TileContext(nc) as tc, ExitStack() as top:
        # ---------- persistent tiles ----------
        cpool = top.enter_context(tc.tile_pool(name="consts", bufs=1))
        ident = cpool.tile([128, 128], BF16)
        nc.sync.dma_start(ident[:], ident_d[:])
        ones2 = cpool.tile([128, 2, 16], FP8)
        nc.sync.dma_start(ones2[:], ones2_d[:])
        ones_row = cpool.tile([1, 128], F32)
        nc.vector.memset(ones_row[:], 1.0)
        eps1 = cpool.tile([1, 1], F32)
        nc.vector.memset(eps1[:], EPS)
        bmix = cpool.tile([128, 3 * MD], F32)
        nc.sync.dma_start(bmix[:], bmix_d[:])

        # out1 (x + h) stays resident in SBUF across phase 1 -> 2b (bf16);
        # normalized out1 resident as fp8 for the FFN matmuls.
        res_pool = top.enter_context(tc.tile_pool(name="resid", bufs=1))
        o1all = res_pool.tile([128, KD, S], BF16)
        o1n = res_pool.tile([128, KD, S], FP8)

        carry_pool = top.enter_context(tc.tile_pool(name="carry", bufs=1))
        carry = carry_pool.tile([128, KD], F32)

        # norm helpers outlive phase 1 (the last two norm2 finishes are
        # emitted between the two FFN gate/up passes)
        p_sq = top.enter_context(tc.tile_pool(name="sqbufs", bufs=2))
        p_row = top.enter_context(tc.tile_pool(name="rows", bufs=1))
        p_bcS = top.enter_context(tc.tile_pool(name="bcS", bufs=1))
        ps_bc = top.enter_context(tc.tile_pool(name="bc_ps", bufs=2, space="PSUM"))

        # ---------- phase 1: mixer (software-pipelined over chunks) ----------
        with ExitStack() as ph1:
            wpool = ph1.enter_context(tc.tile_pool(name="wmix", bufs=1))
            wmix = wpool.tile([128, 2 * MD, KD, 128], FP8)
            wmix_dp = wmix_d.rearrange("m p k j -> p m k j")
            wv16 = wpool.tile([128, MD, KD, 128], BF16)
            wv_dp = wv_d.rearrange("m p k j -> p m k j")

            p_xT = ph1.enter_context(tc.tile_pool(name="xT", bufs=3))
            p_x16 = ph1.enter_context(tc.tile_pool(name="x16", bufs=2))
            p_x8 = ph1.enter_context(tc.tile_pool(name="x8", bufs=2))
            p_16 = ph1.enter_context(tc.tile_pool(name="f16bufs", bufs=2))
            p_scan = ph1.enter_context(tc.tile_pool(name="scanbufs", bufs=1))
            ps_mm = ph1.enter_context(tc.tile_pool(name="mm_ps", bufs=3, space="PSUM"))

            st_front = {}   # c -> (xT, rrow1)
            st_bc1 = {}     # c -> bc1 bf16 SBUF row-broadcast tile
            st_body = {}    # c -> (xs, a_t)  [in-place in sig_g / sig_d]
            st_back = {}    # c -> (out1_slice, rrow2)
            st_sq1 = {}

            def front_t(c):
                """load x chunk transposed via the DMA xbar, squares on GpSimd."""
                s0 = c * CH
                xT = p_xT.tile([128, KD, CH], BF16, tag="xT", name=f"xT{c}")
                nc.sync.dma_start_transpose(xT[:], x_d[s0:s0 + CH, :])
                sq = p_sq.tile([128, KD, CH], FP8, tag="sq", name=f"sq1_{c}")
                nc.scalar.activation(sq[:], xT[:], AF.Square)
                st_front[c] = (xT, None)
                st_sq1[c] = sq

            def norm_rows(sq, label):
                """fp8-DoubleRow ones-reduce + sqrt + fast reciprocal."""
                ss = ps_bc.tile([1, CH], F32, tag="bc", name=f"ss{label}")
                for kp in range(NKP):
                    nc.tensor.matmul(ss[:], ones2[:, :, 0:1],
                                     sq[:, 2 * kp:2 * kp + 2, :],
                                     start=(kp == 0), stop=(kp == NKP - 1),
                                     perf_mode=PM)
                srow = p_row.tile([1, CH], F32, tag=f"srow{label[0]}", name=f"srow{label}")
                nc.scalar.activation(srow[:], ss[:], AF.Sqrt, bias=eps1[:], scale=1.0 / D)
                rrow = p_row.tile([1, CH], F32, tag=f"rrow{label[0]}", name=f"rrow{label}")
                nc.vector.reciprocal_approx_fast(rrow[:], srow[:])
                return rrow

            def bcast(rrow, tag, name):
                """PE f32 row-broadcast -> bf16 SBUF copy (DVE)."""
                bc = ps_bc.tile([128, CH], F32, tag="bc", name=f"bcp_{name}")
                nc.tensor.matmul(bc[:], ones_row[:], rrow[:])
                bcS = p_bcS.tile([128, CH], BF16, tag=tag, name=f"bcS_{name}")
                nc.vector.tensor_copy(bcS[:], bc[:])
                return bcS

            st_ss1 = {}

            def front_red(c):
                sq = st_sq1[c]
                ss = ps_bc.tile([1, CH], F32, tag="bc", name=f"ss1_{c}")
                for kp in range(NKP):
                    nc.tensor.matmul(ss[:], ones2[:, :, 0:1],
                                     sq[:, 2 * kp:2 * kp + 2, :],
                                     start=(kp == 0), stop=(kp == NKP - 1),
                                     perf_mode=PM)
                st_ss1[c] = ss

            def front_sqrt(c):
                ss = st_ss1[c]
                srow = p_row.tile([1, CH], F32, tag="srow1", name=f"srow1_{c}")
                nc.scalar.activation(srow[:], ss[:], AF.Sqrt, bias=eps1[:], scale=1.0 / D)
                rrow = p_row.tile([1, CH], F32, tag="rrow1", name=f"rrow1_{c}")
                nc.vector.reciprocal_approx_fast(rrow[:], srow[:])
                st_front[c] = (st_front[c][0], rrow)

            def bcast1(c):
                st_bc1[c] = bcast(st_front[c][1], "bc1", f"1_{c}")

            st_prep = {}

            def prep(c):
                """normalized input: bf16 (DVE) + fp8 cast (ACT), one chunk
                ahead of body so PE never waits on the normalize chain."""
                xT = st_front[c][0]
                bc1 = st_bc1[c]
                xnTb = p_x16.tile([128, KD, CH], BF16, tag="xnTb", name=f"xnTb{c}")
                for kt in range(KD):
                    nc.vector.tensor_tensor(xnTb[:, kt], xT[:, kt], bc1[:], OP.mult)
                st_prep[c] = (xnTb, None)

            def prep_cast(c):
                xnTb, _ = st_prep[c]
                xnT = p_x8.tile([128, KD, CH], FP8, tag="xnT", name=f"xnT{c}")
                nc.scalar.copy(xnT[:], xnTb[:])
                st_prep[c] = (xnTb, xnT)

            def _proj(c, dst, fn, slot, bcol):
                xnTb, xnT = st_prep[c]
                for half in range(MD // 2):
                    ps = ps_mm.tile([128, 2, CH], F32, tag="mm",
                                    name=f"mm{c}_{slot}_{half}")
                    for mi in range(2):
                        m = half * 2 + mi
                        if slot is None:
                            for kt in range(KD):
                                nc.tensor.matmul(ps[:, mi], wv16[:, m, kt, :],
                                                 xnTb[:, kt, :],
                                                 start=(kt == 0),
                                                 stop=(kt == KD - 1))
                        else:
                            for kp in range(NKP):
                                nc.tensor.matmul(ps[:, mi],
                                                 wmix[:, slot + m, 2 * kp:2 * kp + 2, :],
                                                 xnT[:, 2 * kp:2 * kp + 2, :],
                                                 start=(kp == 0), stop=(kp == NKP - 1),
                                                 perf_mode=PM)
                    for mi in range(2):
                        m = half * 2 + mi
                        nc.scalar.activation(dst[:, m], ps[:, mi], fn,
                                             bias=bmix[:, bcol + m:bcol + m + 1],
                                             scale=(1.0 if slot is None else 1.0 / WS))

            def body_v(c):
                tanh_v = p_16.tile([128, MD, CH], F16, tag="tanh_v", bufs=1, name=f"tv{c}")
                _proj(c, tanh_v, AF.Tanh, None, MD)
                st_body[c] = tanh_v

            def body_gd(c):
                tanh_v = st_body[c]
                sig_g = p_16.tile([128, MD, CH], F16, tag="sig_g", name=f"sg{c}")
                sig_d = p_16.tile([128, MD, CH], F16, tag="sig_d", name=f"sd{c}")
                _proj(c, sig_g, AF.Sigmoid, 0, 0)
                _proj(c, sig_d, AF.Sigmoid, MD, 2 * MD)
                st_body[c] = (sig_g, sig_d, tanh_v)

            def xs_at(c):
                # in-place: xs into sig_g, a_t into sig_d (DVE, after prep so
                # the next chunk's normalize never queues behind these)
                sig_g, sig_d, tanh_v = st_body[c]
                nc.vector.tensor_tensor(sig_g[:], sig_g[:], tanh_v[:], OP.mult)
                nc.vector.tensor_scalar(sig_d[:], sig_d[:], 0.998, 0.001, OP.mult, OP.add)
                st_body[c] = (sig_g, sig_d)

            def back_scan(c):
                """scan, residual into resident out1, squares (DVE+Pool only)."""
                xs, a_t = st_body[c]
                xT = st_front[c][0]
                s0 = c * CH
                hT = p_scan.tile([128, KD, CH], F16, tag="hT", name=f"hT{c}")
                for kt in range(KD):
                    init = 0.0 if c == 0 else carry[:, kt:kt + 1]
                    nc.vector.tensor_tensor_scan(hT[:, kt], a_t[:, kt], xs[:, kt],
                                                 init, OP.mult, OP.add)
                if c + 1 < NCH:
                    nc.vector.tensor_copy(carry[:], hT[:, :, CH - 1])
                out1 = o1all[:, :, s0:s0 + CH]
                nc.vector.tensor_tensor(out1, xT[:], hT[:], OP.add)
                sq2 = p_sq.tile([128, KD, CH], FP8, tag="sq", name=f"sq2_{c}")
                nc.gpsimd.tensor_tensor(sq2[:], out1, out1, OP.mult)
                st_back[c] = (out1, sq2)

            def back_fin(c):
                """norm2 reduce/broadcast (inputs ready); fp8 o1n on GpSimd."""
                s0 = c * CH
                out1, sq2 = st_back[c]
                rrow = norm_rows(sq2, f"2_{c}")
                bcS = bcast(rrow, "bc2", f"2_{c}")
                for kt in range(KD):
                    nc.gpsimd.tensor_tensor(o1n[:, kt, s0:s0 + CH],
                                            out1[:, kt], bcS[:], OP.mult)

            # pipelined emission: prep runs one chunk ahead of body (PE never
            # waits on the normalize chain); norm2 finish lags two chunks.
            front_t(0)
            front_t(1)
            for mt in range(MD):
                nc.sync.dma_start(wv16[:, mt], wv_dp[:, mt])
            for mt in range(2 * MD):
                nc.sync.dma_start(wmix[:, mt], wmix_dp[:, mt])
            front_red(0)
            front_sqrt(0)
            bcast1(0)
            prep(0)
            prep_cast(0)
            for c in range(NCH):
                if 2 <= c + 1 < NCH:
                    front_t(c + 1)
                if c >= 2:
                    back_fin(c - 2)
                body_v(c)
                if c + 1 < NCH:
                    front_red(c + 1)
                    front_sqrt(c + 1)
                    bcast1(c + 1)
                    prep(c + 1)
                body_gd(c)
                if c + 1 < NCH:
                    prep_cast(c + 1)
                xs_at(c)
                if c >= 1:
                    back_scan(c - 1)
            back_scan(NCH - 1)

        # ---------- phase 2: FFN ----------
        with ExitStack() as ph2:
            zpool = ph2.enter_context(tc.tile_pool(name="zbuf", bufs=1))
            z = zpool.tile([128, MFO, S], FP8)
            wo_pool = ph2.enter_context(tc.tile_pool(name="wout", bufs=1))
            wout = wo_pool.tile([128, MD, MFO, 128], FP8)
            wout_dp = wout_d.rearrange("m p k j -> p m k j")
            for mo in range(MD):
                nc.sync.dma_start(wout[:, mo], wout_dp[:, mo])

            # 2a: gate/up + z.  sc=3 deferred so PE need not wait on the
            # final o1n chunk while earlier chunks still have work.
            with ExitStack() as ph2a:
                p_wgu = ph2a.enter_context(tc.tile_pool(name="wgu", bufs=6))
                p_g16 = ph2a.enter_context(tc.tile_pool(name="g16", bufs=3))
                ps_gu = ph2a.enter_context(tc.tile_pool(name="gu_ps", bufs=4, space="PSUM"))
                for scs in ([0, 1], [2, 3]):
                    if scs[0] == 2:
                        back_fin(NCH - 2)
                        back_fin(NCH - 1)
                    for mg in range(MFO):
                        wg = p_wgu.tile([128, KD, 128], FP8, tag="wgu")
                        nc.sync.dma_start(wg[:], wgu_d[mg])
                        wu = p_wgu.tile([128, KD, 128], FP8, tag="wgu")
                        nc.sync.dma_start(wu[:], wgu_d[MFO + mg])
                        for sc in scs:
                            sl = slice(sc * CH, (sc + 1) * CH)
                            gps = ps_gu.tile([128, CH], F32, tag="gups")
                            for kp in range(NKP):
                                nc.tensor.matmul(gps[:], wg[:, 2 * kp:2 * kp + 2, :],
                                                 o1n[:, 2 * kp:2 * kp + 2, sl],
                                                 start=(kp == 0), stop=(kp == NKP - 1),
                                                 perf_mode=PM)
                            ups = ps_gu.tile([128, CH], F32, tag="gups")
                            for kp in range(NKP):
                                nc.tensor.matmul(ups[:], wu[:, 2 * kp:2 * kp + 2, :],
                                                 o1n[:, 2 * kp:2 * kp + 2, sl],
                                                 start=(kp == 0), stop=(kp == NKP - 1),
                                                 perf_mode=PM)
                            gate = p_g16.tile([128, CH], F16, tag="gate")
                            if SILU_ON_ACT:
                                nc.scalar.activation(gate[:], gps[:], AF.Silu,
                                                     scale=1.0 / WS)
                            else:
                                sig = p_g16.tile([128, CH], F16, tag="sig")
                                nc.scalar.activation(sig[:], gps[:], AF.Sigmoid,
                                                     scale=1.0 / WS)
                                nc.vector.scalar_tensor_tensor(gate[:], gps[:], 1.0 / WS,
                                                               sig[:], OP.mult, OP.mult)
                            nc.vector.scalar_tensor_tensor(z[:, mg, sl], ups[:], ZS / WS,
                                                           gate[:], OP.mult, OP.mult)

            # 2b: W_out + residual (from resident out1) + transpose out
            with ExitStack() as ph2b:
                p_oT = ph2b.enter_context(tc.tile_pool(name="outT", bufs=MD + 1))
                p_onat = ph2b.enter_context(tc.tile_pool(name="onat", bufs=3))
                ps_y = ph2b.enter_context(tc.tile_pool(name="y_ps", bufs=2, space="PSUM"))
                ps_t2 = ph2b.enter_context(tc.tile_pool(name="t2_ps", bufs=2, space="PSUM"))
                for sc in range(NCH):
                    sl = slice(sc * CH, (sc + 1) * CH)
                    outTs = []
                    for mo in range(MD):
                        yps = ps_y.tile([128, CH], F32, tag="yps")
                        for op_ in range(NOP):
                            nc.tensor.matmul(yps[:], wout[:, mo, 2 * op_:2 * op_ + 2, :],
                                             z[:, 2 * op_:2 * op_ + 2, sl],
                                             start=(op_ == 0), stop=(op_ == NOP - 1),
                                             perf_mode=PM)
                        oT = p_oT.tile([128, CH], BF16, tag="oT")
                        nc.vector.scalar_tensor_tensor(oT[:], yps[:], 1.0 / (WS * ZS),
                                                       o1all[:, mo, sl], OP.mult, OP.add)
                        outTs.append(oT)
                    for q in range(NST):
                        onat = p_onat.tile([128, D], F32, tag="onat")
                        for h in range(2):
                            t2 = ps_t2.tile([128, 512], BF16, tag="t2")
                            for j in range(4):
                                nc.tensor.transpose(
                                    t2[:, j * 128:(j + 1) * 128],
                                    outTs[4 * h + j][:, q * 128:(q + 1) * 128],
                                    ident[:])
                            nc.scalar.copy(onat[:, h * 512:(h + 1) * 512], t2[:])
                        srow0 = sc * CH + q * 128
                        nc.sync.dma_start(out_d[srow0:srow0 + 128, :], onat[:])

    nc.compile()
    return nc


_NC = None


def _get_nc():
    global _NC
    if _NC is None:
        _NC = build_program()
    return _NC


def _q8(a):
    return np.clip(a, -240.0, 240.0).astype(ml_dtypes.float8_e4m3)


def _prep_weights(inputs):
    w1 = np.asarray(inputs["rms_mix_w"], np.float32)
    w2 = np.asarray(inputs["rms_ffn_w"], np.float32)
    Wg = np.asarray(inputs["Wg"], np.float32) * w1[None, :]
    Wv = np.asarray(inputs["Wv"], np.float32) * w1[None, :]
    Wd = np.asarray(inputs["Wd"], np.float32) * w1[None, :]
    Wcat = np.concatenate([Wg, Wd], axis=0) * WS            # [2D, D] fp8 x256
    w_mix = _q8(np.ascontiguousarray(
        Wcat.T.reshape(KD, 128, 2 * MD, 128).transpose(2, 1, 0, 3)))
    w_v = np.ascontiguousarray(
        Wv.T.reshape(KD, 128, MD, 128).transpose(2, 1, 0, 3)).astype(ml_dtypes.bfloat16)
    bcat = np.concatenate([np.asarray(inputs["bg"], np.float32),
                           np.asarray(inputs["bv"], np.float32),
                           np.asarray(inputs["bd"], np.float32)])
    b_mix = np.ascontiguousarray(bcat.reshape(3 * MD, 128).T).astype(np.float32)
    Wgate = np.asarray(inputs["W_gate"], np.float32) * w2[None, :]
    Wup = np.asarray(inputs["W_up"], np.float32) * w2[None, :]
    Wcat2 = np.concatenate([Wgate, Wup], axis=0) * WS       # [2F, D]
    w_gu = _q8(np.ascontiguousarray(
        Wcat2.T.reshape(KD, 128, MF2, 128).transpose(2, 1, 0, 3)))
    WoT = np.asarray(inputs["W_out"], np.float32).T * WS    # [F, D]
    w_out = _q8(np.ascontiguousarray(
        WoT.reshape(MFO, 128, MD, 128).transpose(2, 1, 0, 3)))
    return {
        "w_mix": w_mix, "w_v": w_v, "b_mix": b_mix, "w_gu": w_gu, "w_out": w_out,
        "ident": np.eye(128).astype(ml_dtypes.bfloat16),
        "ones2": np.ones((128, 2, 16)).astype(ml_dtypes.float8_e4m3),
    }


def run(inputs, trace=False, **kw):
    x = np.asarray(inputs["x"], np.float32)
    shared = _prep_weights(inputs)
    in_maps = [
        dict(shared, x=np.ascontiguousarray(x[b]).astype(ml_dtypes.bfloat16))
        for b in range(B)
    ]
    res = run_bass_kernel_spmd(_get_nc(), in_maps, list(range(B)), trace=trace, **kw)
    out = np.stack([np.asarray(res.results[b]["out"], np.float32) for b in range(B)])
    return out, res


def kernel(**inputs) -> np.ndarray:
    out, _ = run(inputs)
    return out
